# revision 1
# baseline (speedup 1.0000x reference)
"""Multi-head self-attention (B=2, S=2048, D=1024, H=16, causal) on 8 trn2 cores.

Sharding: core c computes heads {2c, 2c+1} for both batches (column-parallel
QKV, row-parallel O). Each core returns a partial [4096, 1024] output
(attention output of its heads projected through its slice of o_proj);
the host sums the 8 partials.

Per-core kernel:
  - host supplies x pre-transposed (xT [1024, 4096]) and per-core weight
    slices pre-laid-out for SBUF.
  - projections (f32r matmuls, xT staged in token-halves): QT/KT
    [128, 2048] per batch stored bf16, V via VT + PE transpose stored bf16
    with a ones column so the AV matmul also produces the softmax
    denominator.
  - attention in transposed-score layout: scoresT[k, q] = K @ Q^T tiles
    (bf16, two heads packed on PE row groups), exp on ACT (scale 1/8
    fused) writing bf16, causal staircase skips invalid columns,
    triangular mask multiplies only diagonal blocks. Lag-1 software
    pipeline: AV for j-1 issues behind scores for j.
  - AV (bf16 in, fp32 accum): avT_aug[65, q] = V_aug^T @ expT; row 64 is
    the denominator. Raw results are copied to SBUF per qc so PSUM frees.
  - normalization (part1): r = exp(-ln(denom)); the denominator row is
    broadcast across 64 partitions with a f32r ones-outer-product matmul,
    ln/exp run on 64 lanes. Head 1 is shifted to partitions 64:128 with
    an SBUF->SBUF DMA so O contracts over all 128 dims in one chain.
  - O projection (part2) in f32r, K=128.
  - scheduling: part1/part2 are deferred and spread into later phases
    (batch 0's part1 into batch 1's projection phase where ACT is idle;
    O matmuls as PE fillers inside later qc j-loops) so the exp stream
    never stalls on the normalize chain.
"""

import os
import numpy as np
from contextlib import ExitStack

import concourse.bass as bass
import concourse.tile as tile
from concourse import bacc, mybir
from concourse.bass_utils import run_bass_kernel_spmd

F32R = mybir.dt.float32r
F32 = mybir.dt.float32
BF16 = mybir.dt.bfloat16
EXP = mybir.ActivationFunctionType.Exp
LN = mybir.ActivationFunctionType.Ln

B, S, D = 2, 2048, 1024
NT = B * S            # 4096 tokens total
NCORES = 8
SCALE = 0.125         # 1/sqrt(64)

_BUILT = None
LAST_RESULTS = None


def _build():
    nc = bacc.Bacc("TRN2", target_bir_lowering=False, debug=False,
                   num_devices=NCORES)
    xt_d = nc.dram_tensor("xt", [D, NT], F32R, kind="ExternalInput").ap()
    wq_d = nc.dram_tensor("wq", [128, D], F32R, kind="ExternalInput").ap()
    wk_d = nc.dram_tensor("wk", [128, D], F32R, kind="ExternalInput").ap()
    wv_d = nc.dram_tensor("wv", [128, D], F32R, kind="ExternalInput").ap()
    wo_d = nc.dram_tensor("wo", [128, 1024], F32R, kind="ExternalInput").ap()
    tri_d = nc.dram_tensor("tri", [128, 128], F32R, kind="ExternalInput").ap()
    id_d = nc.dram_tensor("ident", [128, 128], F32R, kind="ExternalInput").ap()
    ones_d = nc.dram_tensor("ones", [128, 64], F32, kind="ExternalInput").ap()
    out_d = nc.dram_tensor("out", [NT, D], BF16, kind="ExternalOutput").ap()

    with tile.TileContext(nc) as tc, ExitStack() as ctx:
        consts = ctx.enter_context(tc.tile_pool(name="consts", bufs=1))
        sb = ctx.enter_context(tc.tile_pool(name="sb", bufs=1))
        ps = ctx.enter_context(tc.tile_pool(name="ps", bufs=1, space="PSUM"))

        wq_t = consts.tile([128, D], F32R, tag="wq")
        nc.sync.dma_start(wq_t, wq_d)
        wk_t = consts.tile([128, D], F32R, tag="wk")
        nc.sync.dma_start(wk_t, wk_d)
        wv_t = consts.tile([128, D], F32R, tag="wv")
        nc.sync.dma_start(wv_t, wv_d)
        wo_t = consts.tile([128, 1024], F32R, tag="wo")
        nc.sync.dma_start(wo_t, wo_d)
        tri_t = consts.tile([128, 128], BF16, tag="tri")
        nc.gpsimd.dma_start(tri_t, tri_d)   # gpsimd DMA casts f32r->bf16
        id_t = consts.tile([128, 128], F32R, tag="ident")
        nc.sync.dma_start(id_t, id_d)
        # all-ones; row 64 is the lhsT of the f32r broadcast outer-product
        ones_t = consts.tile([65, 64], F32R, tag="ones")
        nc.gpsimd.dma_start(ones_t, ones_d[0:65, 0:64])

        # ---- deferred normalize (part1) and O projection (part2) ----
        def part1(b, qc, rawf):
            """r = exp(-ln(denominator)) broadcast over 64 partitions;
            avt_all[0:64] = h0 normalized, [64:128] = h1 (DMA-shifted)."""
            avt_all = sb.tile([128, 512], F32R, tag="avt", bufs=4,
                              name=f"avt{b}_{qc}")
            scl = sb.tile([128, 512], F32R, tag="scl", bufs=2,
                          name=f"scl{b}_{qc}")
            scl2 = sb.tile([64, 512], F32R, tag="scl2", bufs=2,
                           name=f"scl2_{b}_{qc}")
            lnr = sb.tile([64, 512], F32, tag="lnr", bufs=4,
                          name=f"lnr{b}_{qc}")
            lnr2 = sb.tile([64, 512], F32, tag="lnr", bufs=4,
                           name=f"lnr2{b}_{qc}")
            rawsh = sb.tile([128, 512], F32R, tag="rawsh", bufs=2,
                            name=f"rawsh{b}_{qc}")
            cs = slice(512 * qc, 512 * (qc + 1))
            nc.sync.dma_start(rawsh[64:128, :], rawf[1][0:64, cs])

            bc0 = ps.tile([64, 512], F32, tag="mm", bufs=5,
                          name=f"bc0_{b}_{qc}")
            nc.tensor.matmul(bc0, lhsT=ones_t[64:65, :],
                             rhs=rawf[0][64:65, cs], start=True, stop=True)
            bc1 = ps.tile([64, 512], F32, tag="mm", bufs=5,
                          name=f"bc1_{b}_{qc}")
            nc.tensor.matmul(bc1, lhsT=ones_t[64:65, :],
                             rhs=rawf[1][64:65, cs], start=True, stop=True)
            nc.scalar.activation(lnr[0:64, :], bc0, LN)
            nc.scalar.activation(lnr2[0:64, :], bc1, LN)
            nc.scalar.activation(scl[0:64, :], lnr[0:64, :], EXP, scale=-1.0)
            nc.scalar.activation(scl2, lnr2[0:64, :], EXP, scale=-1.0)
            nc.sync.dma_start(scl[64:128, :], scl2)
            nc.vector.tensor_mul(avt_all[0:64, :], rawf[0][0:64, cs],
                                 scl[0:64, :])
            nc.vector.tensor_mul(avt_all[64:128, :], rawsh[64:128, :],
                                 scl[64:128, :])
            return avt_all

        def part2_unit(b, qc, avt_all, tt):
            """One token-tile of the O projection: 2 matmuls + copy + DMA."""
            ost = sb.tile([128, 1024], BF16, tag="ost", bufs=2,
                          name=f"ost{b}_{qc}_{tt}")
            for chv in range(2):
                op = ps.tile([128, 512], F32, tag="mm", bufs=5,
                             name=f"op{b}_{qc}_{tt}_{chv}")
                nc.tensor.matmul(
                    op,
                    lhsT=avt_all[:, 128 * tt:128 * (tt + 1)],
                    rhs=wo_t[:, 512 * chv:512 * (chv + 1)],
                    start=True, stop=True)
                nc.vector.tensor_copy(ost[:, 512 * chv:512 * (chv + 1)], op)
            row0 = S * b + 512 * qc + 128 * tt
            nc.sync.dma_start(out_d[row0:row0 + 128, :], ost)

        # schedule state
        avt_ready = {}            # (b, qc) -> avt_all tile
        rawf_of = {}              # b -> rawf pair

        def attention(b, qt, kt, vg, rawf, part1_at_j1, fillers):
            """fillers: per-qc list of (b, qc, tt) O-units to spread."""
            for qc in range(4):
                njt = 4 * qc + 4
                avps = [ps.tile([128, 512], F32, tag="av", bufs=2,
                                name=f"avps{b}_{qc}_{h}")
                        for h in range(2)]
                fl = fillers[qc]
                nfl = len(fl)
                p1 = part1_at_j1[qc]
                pend = []

                def do_av(j, ets):
                    vs = max(0, 128 * (j - 4 * qc))
                    for h in range(2):
                        nc.tensor.matmul(
                            avps[h][0:65, vs:512],
                            lhsT=vg[h][:, j, 0:65],
                            rhs=ets[h][:, vs:512],
                            start=(j == 0), stop=(j == njt - 1),
                            skip_group_check=True)

                for j in range(njt):
                    vs = max(0, 128 * (j - 4 * qc))
                    ets = []
                    for h in range(2):
                        sc = ps.tile([128, 512], F32, tag="mm", bufs=5)
                        nc.tensor.matmul(
                            sc[:, vs:512],
                            lhsT=kt[64 * h:64 * (h + 1), 128 * j:128 * (j + 1)],
                            rhs=qt[64 * h:64 * (h + 1), 512 * qc + vs:512 * (qc + 1)],
                            start=True, stop=True)
                        et = sb.tile([128, 512], BF16, tag=f"et{h}", bufs=4)
                        nc.scalar.activation(et[:, vs:512], sc[:, vs:512],
                                             EXP, scale=SCALE)
                        if j >= 4 * qc:
                            nc.vector.tensor_mul(et[:, vs:vs + 128],
                                                 et[:, vs:vs + 128], tri_t)
                        ets.append(et)
                    pend.append((j, ets))
                    if len(pend) > 2:   # lag-2: AV issues two iterations behind
                        do_av(*pend.pop(0))
                    if j == 1 and p1 is not None:
                        avt_ready[p1] = part1(p1[0], p1[1], rawf_of[p1[0]])
                    # spread O-unit fillers across the loop
                    k0 = nfl * j // njt
                    k1 = nfl * (j + 1) // njt
                    for k in range(k0, k1):
                        fb, fqc, ftt = fl[k]
                        part2_unit(fb, fqc, avt_ready[(fb, fqc)], ftt)
                for args in pend:
                    do_av(*args)
                for h in range(2):
                    nc.vector.tensor_copy(rawf[h][:, 512 * qc:512 * (qc + 1)],
                                          avps[h][0:65, :])

        for b in range(B):
            # ---------- projections (token-halves to limit xt residency) --
            xth = []
            for half in range(2):
                row = []
                for k in range(8):
                    xk = sb.tile([128, S // 2], F32R, tag="xt", bufs=9,
                                 name=f"xt{b}_{half}_{k}")
                    nc.sync.dma_start(
                        xk, xt_d[128 * k:128 * (k + 1),
                                 S * b + 1024 * half:S * b + 1024 * (half + 1)])
                    row.append(xk)
                xth.append(row)

            def project(w_t, tag, dt, vbufs=2):
                dst = sb.tile([128, S], dt, tag=tag, bufs=vbufs)
                for chk in range(4):
                    half, sub = chk // 2, chk % 2
                    pp = ps.tile([128, 512], F32, tag="mm", bufs=5)
                    for k in range(8):
                        nc.tensor.matmul(
                            pp, lhsT=w_t[:, 128 * k:128 * (k + 1)],
                            rhs=xth[half][k][:, 512 * sub:512 * (sub + 1)],
                            start=(k == 0), stop=(k == 7))
                    nc.vector.tensor_copy(dst[:, 512 * chk:512 * (chk + 1)], pp)
                return dst

            qt = project(wq_t, "qt", BF16)
            kt = project(wk_t, "kt", BF16)
            vt = project(wv_t, "vt", F32R, vbufs=1)

            # V in token-partition layout, + ones column for the denominator
            vg = []
            for h in range(2):
                vgh = sb.tile([128, 16, 66], BF16, tag=f"vg{h}", bufs=2)
                nc.gpsimd.dma_start(vgh[:, :, 64:65], ones_d[:, 0:16])
                vg.append(vgh)
            for j in range(16):
                tp = ps.tile([128, 128], F32R, tag="mm", bufs=5)
                nc.tensor.transpose(tp, vt[:, 128 * j:128 * (j + 1)], id_t)
                nc.vector.tensor_copy(vg[0][:, j, 0:64], tp[:, 0:64])
                nc.vector.tensor_copy(vg[1][:, j, 0:64], tp[:, 64:128])

            rawf = [sb.tile([65, S], F32R, tag=f"rawfull{h}", bufs=2,
                            name=f"rawf{b}_{h}")
                    for h in range(2)]
            rawf_of[b] = rawf

            if b == 0:
                attention(b, qt, kt, vg, rawf,
                          part1_at_j1=[None] * 4, fillers=[[], [], [], []])
            else:
                # batch 0's normalize + O projection land here: the
                # projection phase has ACT/PE/DVE slack for all of it
                for qc in range(4):
                    avt_ready[(0, qc)] = part1(0, qc, rawf_of[0])
                # b1 attention: all deferred O-units spread evenly (~0.7/j)
                units = ([(0, q, t) for q in range(4) for t in range(4)] +
                         [(1, 0, t) for t in range(4)] +
                         [(1, 1, t) for t in range(4)] +
                         [(1, 2, t) for t in range(4)])
                attention(b, qt, kt, vg, rawf,
                          part1_at_j1=[None, (1, 0), (1, 1), (1, 2)],
                          fillers=[units[0:2], units[2:8], units[8:17],
                                   units[17:28]])
        # tail: the last pieces that cannot hide anywhere
        avt_ready[(1, 3)] = part1(1, 3, rawf_of[1])
        for tt in range(4):
            part2_unit(1, 3, avt_ready[(1, 3)], tt)
    nc.compile()
    return nc


def _get_built():
    global _BUILT
    if _BUILT is None:
        _BUILT = _build()
    return _BUILT


def _host_inputs(x, q_proj, k_proj, v_proj, o_proj):
    xth = np.ascontiguousarray(x.reshape(NT, D).T)
    tri = np.triu(np.ones((128, 128), dtype=np.float32))
    ident = np.eye(128, dtype=np.float32)

    def wslice(w, c):
        # [p, 8k x 128m]: w_sb[p, 128k+m] = w[128c+m, 128k+p]
        a = w[128 * c:128 * (c + 1)].reshape(128, 8, 128)
        return np.ascontiguousarray(a.transpose(2, 1, 0).reshape(128, D))

    in_maps = []
    for c in range(NCORES):
        wo = np.ascontiguousarray(o_proj[:, 128 * c:128 * (c + 1)].T)
        in_maps.append(dict(
            xt=xth, wq=wslice(q_proj, c), wk=wslice(k_proj, c),
            wv=wslice(v_proj, c), wo=wo, tri=tri, ident=ident,
            ones=np.ones((128, 64), dtype=np.float32)))
    return in_maps


def kernel(**inputs):
    x = np.asarray(inputs["x"], dtype=np.float32)
    q_proj = np.asarray(inputs["q_proj"], dtype=np.float32)
    k_proj = np.asarray(inputs["k_proj"], dtype=np.float32)
    v_proj = np.asarray(inputs["v_proj"], dtype=np.float32)
    o_proj = np.asarray(inputs["o_proj"], dtype=np.float32)

    in_maps = _host_inputs(x, q_proj, k_proj, v_proj, o_proj)
    nc = _get_built()
    global LAST_RESULTS
    LAST_RESULTS = run_bass_kernel_spmd(
        nc, in_maps, core_ids=list(range(NCORES)),
        trace=bool(os.environ.get("KERNEL_TRACE")))
    acc = np.asarray(LAST_RESULTS.results[0]["out"]).astype(np.float32)
    for c in range(1, NCORES):
        acc += np.asarray(LAST_RESULTS.results[c]["out"]).astype(np.float32)
    return acc.reshape(B, S, D)



# revision 3
# speedup vs baseline: 1.1398x; 1.1398x over previous
"""Multi-head self-attention (B=2, S=2048, D=1024, H=16, causal) on 8 trn2 cores.

Sharding: batch x heads. Core c owns batch c//4 and heads
[4*(c%4), 4*(c%4)+4) as two head-pairs. Each core returns a partial
[2048, 1024] output (its heads' attention projected through its slice
of o_proj); the host sums 4 partials per batch.

Per-core kernel (all operands bf16, f32 PSUM accumulation):
  - x arrives pre-transposed and bf16-cast (xt [1024, 2048]), loaded
    once into 8 SBUF k-tiles and reused by both head-pairs'
    projections.
  - projections per pair: qt/kt [128, 2048] bf16 (2 heads stacked on
    partitions); vt transposed via PE into vg [tokens, j, 66] with a
    ones column so the AV matmul also produces the softmax denominator.
  - scores in transposed layout scT[k, q] = K @ Q^T; the two heads'
    64-contraction matmuls are emitted back-to-back so they co-execute
    in PE row groups (0,0)/(64,0). exp on ACT (scale 1/8 fused) writes
    bf16; causal staircase skips invalid columns; triangular mask
    multiplies only diagonal blocks.
  - AV in normal orientation per (head, 128-query tile): av[q, 65]
    accumulates over key tiles in PSUM; column 64 is the denominator.
  - normalize: DVE reciprocal of the denominator column + ACT
    scale-copy (per-partition scale) -> av_n bf16; PE transpose packs
    both heads into avt[:, qtile] (128 head-dims x 128 queries).
  - O projection per token-tile accumulates both head-pairs in one
    PSUM group; emitted lag-1 behind pair 1's attention so the tail is
    a single token-tile.
"""

import os
import numpy as np
from contextlib import ExitStack

import ml_dtypes

import concourse.bass as bass
import concourse.tile as tile
from concourse import bacc, mybir
from concourse.bass_utils import run_bass_kernel_spmd

F32 = mybir.dt.float32
BF16 = mybir.dt.bfloat16
EXP = mybir.ActivationFunctionType.Exp

B, S, D = 2, 2048, 1024
NCORES = 8
SCALE = 0.125          # 1/sqrt(64)
NQT = S // 128         # 16 query tiles per core
BF = ml_dtypes.bfloat16

_BUILT = None
LAST_RESULTS = None


def _build():
    nc = bacc.Bacc("TRN2", target_bir_lowering=False, debug=False,
                   num_devices=NCORES)
    xt_d = nc.dram_tensor("xt", [D, S], BF16, kind="ExternalInput").ap()
    wq_d = nc.dram_tensor("wq", [2, 128, D], BF16, kind="ExternalInput").ap()
    wk_d = nc.dram_tensor("wk", [2, 128, D], BF16, kind="ExternalInput").ap()
    wv_d = nc.dram_tensor("wv", [2, 128, D], BF16, kind="ExternalInput").ap()
    wo_d = nc.dram_tensor("wo", [2, 128, D], BF16, kind="ExternalInput").ap()
    tri_d = nc.dram_tensor("tri", [128, 128], BF16, kind="ExternalInput").ap()
    id_d = nc.dram_tensor("ident", [128, 128], BF16, kind="ExternalInput").ap()
    out_d = nc.dram_tensor("out", [S, D], BF16, kind="ExternalOutput").ap()

    with tile.TileContext(nc) as tc, ExitStack() as ctx:
        consts = ctx.enter_context(tc.tile_pool(name="consts", bufs=1))
        sb = ctx.enter_context(tc.tile_pool(name="sb", bufs=1))
        ps = ctx.enter_context(tc.tile_pool(name="ps", bufs=1, space="PSUM"))

        # weights for pair 0 first, then x, then the rest: the first
        # projection matmul only waits for wq0 + xth[0]
        wq_t = [consts.tile([128, D], BF16, tag="wq", bufs=2, name=f"wq{p}")
                for p in range(2)]
        wk_t = [consts.tile([128, D], BF16, tag="wk", bufs=2, name=f"wk{p}")
                for p in range(2)]
        wv_t = [consts.tile([128, D], BF16, tag="wv", bufs=2, name=f"wv{p}")
                for p in range(2)]
        wo_t = [consts.tile([128, D], BF16, tag="wo", bufs=2, name=f"wo{p}")
                for p in range(2)]
        tri_t = consts.tile([128, 128], BF16, tag="tri")
        id_t = consts.tile([128, 128], BF16, tag="ident")

        nc.sync.dma_start(wq_t[0], wq_d[0])
        xth = []
        for k in range(8):
            xk = sb.tile([128, S], BF16, tag="xt", bufs=8, name=f"xt{k}")
            nc.sync.dma_start(xk, xt_d[128 * k:128 * (k + 1), :])
            xth.append(xk)
        nc.sync.dma_start(wk_t[0], wk_d[0])
        nc.sync.dma_start(wv_t[0], wv_d[0])
        nc.gpsimd.dma_start(tri_t, tri_d)
        nc.gpsimd.dma_start(id_t, id_d)
        nc.sync.dma_start(wq_t[1], wq_d[1])
        nc.sync.dma_start(wk_t[1], wk_d[1])
        nc.sync.dma_start(wv_t[1], wv_d[1])
        nc.sync.dma_start(wo_t[0], wo_d[0])
        nc.sync.dma_start(wo_t[1], wo_d[1])

        qt = [None, None]
        kt = [None, None]
        vg = [[None, None], [None, None]]
        avt = [None, None]

        def project(p):
            def one(w_t, tag):
                dst = sb.tile([128, S], BF16, tag=tag, bufs=2,
                              name=f"{tag}{p}")
                for chk in range(4):
                    pp = ps.tile([128, 512], F32, tag="sc", bufs=3)
                    for k in range(8):
                        nc.tensor.matmul(
                            pp, lhsT=w_t[:, 128 * k:128 * (k + 1)],
                            rhs=xth[k][:, 512 * chk:512 * (chk + 1)],
                            start=(k == 0), stop=(k == 7))
                    nc.scalar.copy(dst[:, 512 * chk:512 * (chk + 1)], pp)
                return dst

            qt[p] = one(wq_t[p], "qt")
            kt[p] = one(wk_t[p], "kt")
            vt = one(wv_t[p], "vt")
            for h in range(2):
                vgh = sb.tile([128, NQT, 66], BF16, tag=f"vg{h}", bufs=2,
                              name=f"vg{p}_{h}")
                nc.gpsimd.memset(vgh[:, :, 64:65], 1.0)
                vg[p][h] = vgh
            for j in range(NQT):
                tp = ps.tile([128, 128], BF16, tag="tp", bufs=1)
                nc.tensor.transpose(tp, vt[:, 128 * j:128 * (j + 1)], id_t)
                nc.vector.tensor_copy(vg[p][0][:, j, 0:64], tp[:, 0:64])
                nc.vector.tensor_copy(vg[p][1][:, j, 0:64], tp[:, 64:128])

        def o_unit(tt):
            ost = sb.tile([128, D], BF16, tag="ost", bufs=2, name=f"ost{tt}")
            for chv in range(2):
                op = ps.tile([128, 512], F32, tag="sc", bufs=3)
                nc.tensor.matmul(
                    op, lhsT=avt[0][:, 128 * tt:128 * (tt + 1)],
                    rhs=wo_t[0][:, 512 * chv:512 * (chv + 1)],
                    start=True, stop=False, skip_group_check=True)
                nc.tensor.matmul(
                    op, lhsT=avt[1][:, 128 * tt:128 * (tt + 1)],
                    rhs=wo_t[1][:, 512 * chv:512 * (chv + 1)],
                    start=False, stop=True, skip_group_check=True)
                nc.scalar.copy(ost[:, 512 * chv:512 * (chv + 1)], op)
            nc.sync.dma_start(out_d[128 * tt:128 * (tt + 1), :], ost)

        def attention(p, fillers):
            """fillers[qi]: list of thunks to emit inside qtile qi's loop."""
            avt[p] = sb.tile([128, S], BF16, tag="avt", bufs=2,
                             name=f"avt{p}")
            et_of = {}
            cur_qc = [-1]

            def ensure_scores(qc, j):
                if cur_qc[0] != qc:
                    et_of.clear()
                    cur_qc[0] = qc
                if j in et_of:
                    return et_of[j]
                vs = max(0, 128 * (j - 4 * qc))
                scs = []
                for h in range(2):
                    sc = ps.tile([128, 512], F32, tag="sc", bufs=3)
                    nc.tensor.matmul(
                        sc[:, vs:512],
                        lhsT=kt[p][64 * h:64 * (h + 1),
                                   128 * j:128 * (j + 1)],
                        rhs=qt[p][64 * h:64 * (h + 1),
                                  512 * qc + vs:512 * (qc + 1)],
                        start=True, stop=True)
                    scs.append(sc)
                ets = []
                for h in range(2):
                    et = sb.tile([128, 512], BF16, tag=f"et{h}", bufs=18)
                    nc.scalar.activation(et[:, vs:512], scs[h][:, vs:512],
                                         EXP, scale=SCALE)
                    if j >= 4 * qc:
                        nc.vector.tensor_mul(et[:, vs:vs + 128],
                                             et[:, vs:vs + 128], tri_t)
                    ets.append(et)
                et_of[j] = ets
                return ets

            for qi in range(NQT):
                qc, s = qi // 4, qi % 4
                avps = [ps.tile([128, 65], F32, tag="av", bufs=4,
                                name=f"av{p}_{qi}_{h}")
                        for h in range(2)]
                for fl in fillers[qi]:
                    fl()
                for j in range(qi + 1):
                    ets = ensure_scores(qc, j)
                    for h in range(2):
                        nc.tensor.matmul(
                            avps[h],
                            lhsT=ets[h][:, 128 * s:128 * (s + 1)],
                            rhs=vg[p][h][:, j, 0:65],
                            start=(j == 0), stop=(j == qi),
                            skip_group_check=True)
                # normalize + transpose into avt
                tp = ps.tile([128, 128], BF16, tag="tp", bufs=1)
                for h in range(2):
                    r = sb.tile([128, 1], F32, tag="r", bufs=4)
                    nc.vector.reciprocal(r, avps[h][:, 64:65])
                    av_n = sb.tile([128, 64], BF16, tag="avn", bufs=4)
                    nc.scalar.mul(av_n, avps[h][:, 0:64], r)
                    nc.tensor.transpose(tp[64 * h:64 * (h + 1), :],
                                        av_n, id_t)
                nc.vector.tensor_copy(avt[p][:, 128 * qi:128 * (qi + 1)], tp)

        project(0)
        project(1)
        attention(0, fillers=[[] for _ in range(NQT)])
        # pair-1 attention carries the O projection, lag-1 behind its
        # own qtile completions (O(tt) needs avt[1][:, tt])
        attention(1, fillers=[[] if qi == 0 else
                              [lambda tt=qi - 1: o_unit(tt)]
                              for qi in range(NQT)])
        o_unit(NQT - 1)
    nc.compile()
    return nc


def _get_built():
    global _BUILT
    if _BUILT is None:
        _BUILT = _build()
    return _BUILT


def _host_inputs(x, q_proj, k_proj, v_proj, o_proj):
    tri = np.triu(np.ones((128, 128), dtype=np.float32)).astype(BF)
    ident = np.eye(128, dtype=np.float32).astype(BF)
    xt = [np.ascontiguousarray(x[b].T).astype(BF) for b in range(B)]

    def wslice(w, gp):
        # [p, 8k x 128m]: w_sb[p, 128k+m] = w[128gp+m, 128k+p]
        a = w[128 * gp:128 * (gp + 1)].reshape(128, 8, 128)
        return np.ascontiguousarray(a.transpose(2, 1, 0).reshape(128, D))

    in_maps = []
    for c in range(NCORES):
        b, g4 = c // 4, c % 4
        gps = (2 * g4, 2 * g4 + 1)
        wq = np.stack([wslice(q_proj, gp) for gp in gps]).astype(BF)
        wk = np.stack([wslice(k_proj, gp) for gp in gps]).astype(BF)
        wv = np.stack([wslice(v_proj, gp) for gp in gps]).astype(BF)
        wo = np.stack(
            [np.ascontiguousarray(o_proj[:, 128 * gp:128 * (gp + 1)].T)
             for gp in gps]).astype(BF)
        in_maps.append(dict(xt=xt[b], wq=wq, wk=wk, wv=wv, wo=wo,
                            tri=tri, ident=ident))
    return in_maps


def kernel(**inputs):
    x = np.asarray(inputs["x"], dtype=np.float32)
    q_proj = np.asarray(inputs["q_proj"], dtype=np.float32)
    k_proj = np.asarray(inputs["k_proj"], dtype=np.float32)
    v_proj = np.asarray(inputs["v_proj"], dtype=np.float32)
    o_proj = np.asarray(inputs["o_proj"], dtype=np.float32)

    in_maps = _host_inputs(x, q_proj, k_proj, v_proj, o_proj)
    nc = _get_built()
    global LAST_RESULTS
    LAST_RESULTS = run_bass_kernel_spmd(
        nc, in_maps, core_ids=list(range(NCORES)),
        trace=bool(os.environ.get("KERNEL_TRACE")))
    y = np.zeros((B, S, D), dtype=np.float32)
    for c in range(NCORES):
        y[c // 4] += np.asarray(LAST_RESULTS.results[c]["out"]).astype(
            np.float32)
    return y
